# revision 20
# baseline (speedup 1.0000x reference)
"""CornerPool module kernel for Trainium2 (Bass/Tile), 8-core batch-parallel.

Model (per sample, C=256, H=W=128):
  t = relu(bn(conv3x3(x, w_t)));  tp = reverse-cummax_H(t)
  l = relu(bn(conv3x3(x, w_l)));  lp = reverse-cummax_W(l)
  b = relu(bn(conv3x3(x, w_b)));  bp = cummax_H(b)
  r = relu(bn(conv3x3(x, w_r)));  rp = cummax_W(r)
  tl = relu(bn3(conv3x3(tp+lp)) + bn1(conv1x1(x)));  out_tl = relu(bn(conv3x3(tl)))
  br = relu(bn3(conv3x3(bp+rp)) + bn1(conv1x1(x)));  out_br = relu(bn(conv3x3(br)))

Strategy: one sample per NeuronCore (B=8). All convs lowered to bf16
matmuls (full-rate, small LDWEIGHTS that hides under the 512-row stream)
over 128-channel tiles with N=512 (4 image rows) PSUM accumulation
groups; BN scale folded into weights on host, bias applied in the
ScalarE relu epilogue which also downcasts activations to bf16 for the
next stage. Corner pools: H-direction via 2-step shifted-max doubling +
inter-strip carry, W-direction via the native DVE prefix-scan
instruction (per image row, reversed AP for left-pool). Intermediates
(pooled maps, tp+lp sums, tl/br) round-trip through padded internal-DRAM
scratch in bf16 so every 3x3 conv reads zero-padded halos uniformly.
"""

import numpy as np
import ml_dtypes

_P = 128          # partitions / channel tile
_SR = 4           # image rows per strip (N = _SR*128 = 512)
_BF = ml_dtypes.bfloat16


def _prep_host(inputs):
    """Fold BN scales into weights, build lhsT-layout bf16 weight arrays and
    the combined f32 bias table. Returns dict of shared input arrays."""
    f32 = np.float32

    def scaled(name):
        w = np.asarray(inputs["w_" + name], f32)
        s = np.asarray(inputs["s_" + name], f32)
        return w * s[:, None, None, None]

    def bias(name):
        return np.asarray(inputs["b_" + name], f32)

    # stage A convs: [128co, 256ci, 3, 3] -> [128k, 18(ci_t*9+dydx), 128m]
    def layA(w):
        a = w.transpose(1, 2, 3, 0).reshape(2, 128, 9, 128)   # ci_t,k,dydx,m
        return np.ascontiguousarray(a.transpose(1, 0, 2, 3).reshape(128, 18, 128))

    wa = np.stack([layA(scaled(n)) for n in ("t", "l", "b", "r")])  # [4,128,18,128]

    # stage C: w3 [256co,128ci,3,3] -> [k, co_t*9+dydx, m];
    #          w1 [256co,256ci,1,1] -> [k, co_t*2+ci_t, m]; concat -> 22 slices
    def layC(w3, w1):
        a3 = w3.transpose(1, 2, 3, 0).reshape(128, 9, 2, 128)     # k,dydx,co_t,m
        a3 = a3.transpose(0, 2, 1, 3).reshape(128, 18, 128)
        a1 = w1[:, :, 0, 0].T.reshape(2, 128, 2, 128)             # ci_t,k,co_t,m
        a1 = a1.transpose(1, 2, 0, 3).reshape(128, 4, 128)        # k, co_t*2+ci_t, m
        return np.ascontiguousarray(np.concatenate([a3, a1], axis=1))

    wc = np.stack([layC(scaled("tl3"), scaled("tl1")),
                   layC(scaled("br3"), scaled("br1"))])            # [2,128,22,128]

    # stage D: [256co,256ci,3,3] -> [k, co_t, ci_t*9+dydx, m]
    def layD(w):
        a = w.transpose(1, 2, 3, 0).reshape(2, 128, 3, 3, 2, 128)  # ci_t,k,dy,dx,co_t,m
        a = a.transpose(1, 4, 0, 2, 3, 5).reshape(128, 2, 18, 128)
        return np.ascontiguousarray(a)

    wd = np.stack([layD(scaled("tlo")), layD(scaled("bro"))])      # [2,128,2,18,128]

    bias_rows = [bias("t"), bias("l"), bias("b"), bias("r")]       # 0..3
    for bi, (n3, n1) in enumerate((("tl3", "tl1"), ("br3", "br1"))):
        comb = bias(n3) + bias(n1)                                 # [256]
        bias_rows += [comb[:128], comb[128:]]                      # 4+bi*2+co_t
    for n in ("tlo", "bro"):
        bb = bias(n)
        bias_rows += [bb[:128], bb[128:]]                          # 8+bi*2+co_t
    bias_all = np.ascontiguousarray(np.stack(bias_rows).T).astype(f32)  # [128,12]

    return {"wa": wa.astype(_BF), "wc": wc.astype(_BF),
            "wd": wd.astype(_BF), "bias": bias_all}


def _pad_x_sample(xs, H):
    """[256,H,128] f32 -> [2,128,H+2,130] zero-padded bf16."""
    xp = np.zeros((2, 128, H + 2, 130), _BF)
    xp[:, :, 1:H + 1, 1:129] = np.asarray(xs, np.float32).reshape(
        2, 128, H, 128).astype(_BF)
    return xp


def _build(H):
    """Build the Bass module for one core (one sample of height H)."""
    import concourse.bacc as bacc
    import concourse.mybir as mybir
    import concourse.tile as tile

    dt = mybir.dt
    Alu = mybir.AluOpType
    Act = mybir.ActivationFunctionType
    S = H // _SR
    HP = H + 2
    NPIX = HP * 130

    nc = bacc.Bacc("TRN2", target_bir_lowering=False, debug=False)

    xpad = nc.dram_tensor("xpad", [2, 128, HP, 130], dt.bfloat16, kind="ExternalInput")
    wa_d = nc.dram_tensor("wa", [4, 128, 18, 128], dt.bfloat16, kind="ExternalInput")
    wc_d = nc.dram_tensor("wc", [2, 128, 22, 128], dt.bfloat16, kind="ExternalInput")
    wd_d = nc.dram_tensor("wd", [2, 128, 2, 18, 128], dt.bfloat16, kind="ExternalInput")
    bias_d = nc.dram_tensor("bias", [128, 12], dt.float32, kind="ExternalInput")
    out_tl = nc.dram_tensor("out_tl", [256, H, 128], dt.bfloat16, kind="ExternalOutput")
    out_br = nc.dram_tensor("out_br", [256, H, 128], dt.bfloat16, kind="ExternalOutput")

    # internal DRAM scratch (bf16, produced rounded on-chip)
    tp_d = nc.dram_tensor("tp_s", [128, H, 128], dt.bfloat16)
    bp_d = nc.dram_tensor("bp_s", [128, H, 128], dt.bfloat16)
    sum_d = nc.dram_tensor("sum_s", [2, 128, HP, 130], dt.bfloat16)
    tlb_d = nc.dram_tensor("tlb_s", [2, 2, 128, HP, 130], dt.bfloat16)

    with tile.TileContext(nc) as tc:
        import contextlib
        with contextlib.ExitStack() as ctx:
            xpool = ctx.enter_context(tc.tile_pool(name="xp", bufs=1))
            wpool = ctx.enter_context(tc.tile_pool(name="wp", bufs=2))
            spool = ctx.enter_context(tc.tile_pool(name="sp", bufs=2))
            opool = ctx.enter_context(tc.tile_pool(name="op", bufs=2))
            wpool2 = ctx.enter_context(tc.tile_pool(name="wide", bufs=2))
            hpool = ctx.enter_context(tc.tile_pool(name="hp", bufs=2))
            cpool = ctx.enter_context(tc.tile_pool(name="cp", bufs=2))
            mpool = ctx.enter_context(tc.tile_pool(name="mp", bufs=1))
            pspool = ctx.enter_context(tc.tile_pool(name="ps", bufs=8, space="PSUM"))

            # ---- PE warm-up: ramp the p-state while the first DMAs land
            wwarm = mpool.tile([128, 128], dt.bfloat16, tag="warmw")
            xwarm = mpool.tile([128, 512], dt.bfloat16, tag="warmx")
            nc.vector.memset(wwarm[:], 0.0)
            nc.vector.memset(xwarm[:], 0.0)
            pswarm = pspool.tile([128, 512], dt.float32, tag="ps")
            for i in range(12):
                nc.tensor.matmul(pswarm[:], wwarm[:], xwarm[:],
                                 start=(i == 0), stop=(i == 11))

            # ---- preamble: x, biases, zero borders --------------------
            # first chunk small (8 rows) so the first conv group starts fast
            nch = 8
            r0 = HP - 8
            bounds = [HP] + [r0 - (r0 * k) // (nch - 1) for k in range(nch)]
            xt0 = xpool.tile([128, NPIX], dt.bfloat16, tag="x0")
            xt1 = xpool.tile([128, NPIX], dt.bfloat16, tag="x1")
            xt = [xt0, xt1]

            def load_x_chunk(k, engs=(None, None)):
                for ci, eng in ((0, engs[0] or nc.sync),
                                (1, engs[1] or nc.scalar)):
                    a, b = bounds[k + 1], bounds[k]
                    seg = xt[ci][:, a * 130:b * 130]
                    eng.dma_start(seg, xpad.ap()[ci][:, a:b, :])

            xr = [t[:].rearrange("p (a b) -> p a b", b=130) for t in xt]

            def load_w(src_ap, nsl):
                t = wpool.tile([128, nsl, 128], dt.bfloat16, tag="w")
                h = nsl // 2
                nc.sync.dma_start(t[:, :h], src_ap[:, :h])
                nc.scalar.dma_start(t[:, h:], src_ap[:, h:])
                return t

            # first conv needs only w_t slice 0 and the top x chunk; make
            # slice 0 a tiny DMA so the first matmul group starts sooner
            w_t = wpool.tile([128, 18, 128], dt.bfloat16, tag="w")
            nc.sync.dma_start(w_t[:, :1], wa_d.ap()[0][:, :1])
            nc.scalar.dma_start(w_t[:, 1:9], wa_d.ap()[0][:, 1:9])
            load_x_chunk(0, engs=(nc.gpsimd, nc.sync))
            nc.scalar.dma_start(w_t[:, 9:], wa_d.ap()[0][:, 9:])

            bt = mpool.tile([128, 12], dt.float32, tag="bias")
            nc.gpsimd.dma_start(bt[:], bias_d.ap())
            for _k in range(1, nch):
                load_x_chunk(_k)

            def conv_a_mms(ps, w, s):
                i = 0
                for ci in range(2):
                    for dy in range(3):
                        for dx in range(3):
                            nc.tensor.matmul(
                                ps[:], w[:, ci * 9 + dy * 3 + dx],
                                xr[ci][:, _SR * s + dy:_SR * s + dy + _SR,
                                       dx:dx + 128],
                                start=(i == 0), stop=(i == 17))
                            i += 1

            def act_half(t2, half, ps, brow):
                # ACT into one 4-row half of a 2-strip [128, 8, 128] tile
                h0 = half * _SR
                nc.scalar.activation(
                    t2[:, h0:h0 + _SR].rearrange("p a b -> p (a b)"), ps[:],
                    Act.Relu, bias=bt[:, brow:brow + 1], scale=1.0)

            def act_strip2(ps, brow):
                t = spool.tile([128, _SR, 128], dt.bfloat16, tag="cb")
                nc.scalar.activation(t[:].rearrange("p a b -> p (a b)"), ps[:],
                                     Act.Relu, bias=bt[:, brow:brow + 1],
                                     scale=1.0)
                return t

            def wide_tile():
                # [128, 8, 130] with zeroed w-border columns
                t = wpool2.tile([128, 2 * _SR, 130], dt.bfloat16, tag="cw")
                nc.gpsimd.memset(t[:, :, 0:1], 0.0)
                nc.gpsimd.memset(t[:, :, 129:130], 0.0)
                return t

            def act_half_wide(t2, half, ps, brow):
                h0 = half * _SR
                nc.scalar.activation(t2[:, h0:h0 + _SR, 1:129], ps[:],
                                     Act.Relu, bias=bt[:, brow:brow + 1],
                                     scale=1.0)

            # ---- pass T: conv t, reverse cummax over H (strips desc) --
            zt = mpool.tile([128, 130], dt.bfloat16, tag="zero")
            nc.vector.memset(zt[:], 0.0)
            for i, buf in enumerate((sum_d.ap()[0], sum_d.ap()[1],
                                     tlb_d.ap()[0, 0], tlb_d.ap()[0, 1],
                                     tlb_d.ap()[1, 0], tlb_d.ap()[1, 1])):
                eng = nc.sync if i % 2 else nc.scalar
                eng.dma_start(buf[:, 0, :], zt[:, :130])
                eng.dma_start(buf[:, HP - 1, :], zt[:, :130])

            carry = cpool.tile([128, 1, 128], dt.bfloat16, tag="cryT")
            nc.vector.memset(carry[:], 0.0)
            for sb in reversed(range(S // 2)):
                ct2 = spool.tile([128, 2 * _SR, 128], dt.bfloat16, tag="ct")
                for half in (1, 0):                       # strips desc
                    s = 2 * sb + half
                    h0 = half * _SR
                    ps = pspool.tile([128, 512], dt.float32, tag="ps")
                    conv_a_mms(ps, w_t, s)
                    act_half(ct2, half, ps, 0)
                    ct = ct2[:, h0:h0 + _SR]
                    nc.vector.tensor_tensor(ct2[:, h0:h0 + 3], ct2[:, h0:h0 + 3],
                                            ct2[:, h0 + 1:h0 + 4], Alu.max)
                    nc.vector.tensor_tensor(ct2[:, h0:h0 + 2], ct2[:, h0:h0 + 2],
                                            ct2[:, h0 + 2:h0 + 4], Alu.max)
                    nc.vector.tensor_tensor(ct, ct,
                                            carry[:].broadcast_to([128, _SR, 128]),
                                            Alu.max)
                    if s != 0:
                        nxt = cpool.tile([128, 1, 128], dt.bfloat16, tag="cryT")
                        nc.vector.tensor_copy(nxt[:], ct2[:, h0:h0 + 1])
                        carry = nxt
                nc.sync.dma_start(
                    tp_d.ap()[:, 2 * _SR * sb:2 * _SR * (sb + 1), :], ct2[:])

            # ---- pass B: conv b, forward cummax over H (asc) ----------
            w_b = load_w(wa_d.ap()[2], 18)
            carry = cpool.tile([128, 1, 128], dt.bfloat16, tag="cryB")
            nc.vector.memset(carry[:], 0.0)
            for sb in range(S // 2):
                p2 = spool.tile([128, 2 * _SR, 128], dt.bfloat16, tag="p1")
                for half in (0, 1):                       # strips asc
                    s = 2 * sb + half
                    h0 = half * _SR
                    ps = pspool.tile([128, 512], dt.float32, tag="ps")
                    conv_a_mms(ps, w_b, s)
                    ct = act_strip2(ps, 2)
                    p1 = p2[:, h0:h0 + _SR]
                    nc.vector.tensor_tensor(p2[:, h0 + 1:h0 + 4], ct[:, 1:4],
                                            ct[:, 0:3], Alu.max)
                    nc.vector.tensor_copy(p2[:, h0:h0 + 1], ct[:, 0:1])
                    nc.vector.tensor_tensor(p2[:, h0 + 2:h0 + 4],
                                            p2[:, h0 + 2:h0 + 4],
                                            p2[:, h0:h0 + 2], Alu.max)
                    nc.vector.tensor_tensor(p1, p1,
                                            carry[:].broadcast_to([128, _SR, 128]),
                                            Alu.max)
                    if s != S - 1:
                        nxt = cpool.tile([128, 1, 128], dt.bfloat16, tag="cryB")
                        nc.vector.tensor_copy(nxt[:], p2[:, h0 + 3:h0 + 4])
                        carry = nxt
                nc.sync.dma_start(
                    bp_d.ap()[:, 2 * _SR * sb:2 * _SR * (sb + 1), :], p2[:])

            # ---- pass L: conv l, reverse cummax over W, add tp --------
            w_l = load_w(wa_d.ap()[1], 18)
            for sb in range(S // 2):
                ct2 = wide_tile()
                tps = spool.tile([128, 2 * _SR, 128], dt.bfloat16, tag="tps")
                nc.sync.dma_start(
                    tps[:], tp_d.ap()[:, 2 * _SR * sb:2 * _SR * (sb + 1), :])
                for half in (0, 1):
                    s = 2 * sb + half
                    ps = pspool.tile([128, 512], dt.float32, tag="ps")
                    conv_a_mms(ps, w_l, s)
                    act_half_wide(ct2, half, ps, 1)
                    for h in range(_SR):
                        v = ct2[:, half * _SR + h, 1:129][:, ::-1]
                        nc.vector.tensor_tensor_scan(v, v, v, 0.0,
                                                     op0=Alu.max, op1=Alu.bypass)
                nc.vector.tensor_tensor(ct2[:, :, 1:129], ct2[:, :, 1:129],
                                        tps[:], Alu.add)
                nc.sync.dma_start(
                    sum_d.ap()[0][:, 1 + 2 * _SR * sb:1 + 2 * _SR * (sb + 1), :],
                    ct2[:])

            # ---- pass R: conv r, forward cummax over W, add bp --------
            w_r = load_w(wa_d.ap()[3], 18)
            for sb in range(S // 2):
                ct2 = wide_tile()
                tps = spool.tile([128, 2 * _SR, 128], dt.bfloat16, tag="tps")
                nc.sync.dma_start(
                    tps[:], bp_d.ap()[:, 2 * _SR * sb:2 * _SR * (sb + 1), :])
                for half in (0, 1):
                    s = 2 * sb + half
                    ps = pspool.tile([128, 512], dt.float32, tag="ps")
                    conv_a_mms(ps, w_r, s)
                    act_half_wide(ct2, half, ps, 3)
                    for h in range(_SR):
                        v = ct2[:, half * _SR + h, 1:129]
                        nc.vector.tensor_tensor_scan(v, v, v, 0.0,
                                                     op0=Alu.max, op1=Alu.bypass)
                nc.vector.tensor_tensor(ct2[:, :, 1:129], ct2[:, :, 1:129],
                                        tps[:], Alu.add)
                nc.sync.dma_start(
                    sum_d.ap()[1][:, 1 + 2 * _SR * sb:1 + 2 * _SR * (sb + 1), :],
                    ct2[:])

            # ---- stage C: tl = relu(conv3x3(sum) + conv1x1(x)) --------
            for bi in range(2):
                w_c = load_w(wc_d.ap()[bi], 22)
                for sb in range(S // 2):
                    sums = hpool.tile([128, 10, 130], dt.bfloat16, tag="sums")
                    nc.sync.dma_start(
                        sums[:],
                        sum_d.ap()[bi][:, 2 * _SR * sb:2 * _SR * sb + 10, :])
                    for co in range(2):
                        cst2 = wide_tile()
                        for half in (0, 1):
                            s = 2 * sb + half
                            off = half * _SR
                            ps = pspool.tile([128, 512], dt.float32, tag="ps")
                            i = 0
                            for dy in range(3):
                                for dx in range(3):
                                    nc.tensor.matmul(
                                        ps[:], w_c[:, co * 9 + dy * 3 + dx],
                                        sums[:, off + dy:off + dy + _SR,
                                             dx:dx + 128],
                                        start=(i == 0), stop=False)
                                    i += 1
                            for ci in range(2):
                                nc.tensor.matmul(
                                    ps[:], w_c[:, 18 + co * 2 + ci],
                                    xr[ci][:, 1 + _SR * s:1 + _SR * (s + 1),
                                           1:129],
                                    start=False, stop=(ci == 1))
                            act_half_wide(cst2, half, ps, 4 + bi * 2 + co)
                        nc.sync.dma_start(
                            tlb_d.ap()[bi, co][:, 1 + 2 * _SR * sb:
                                               1 + 2 * _SR * (sb + 1), :],
                            cst2[:])

            # ---- stage D: out = relu(conv3x3(tl)) ---------------------
            for bi in range(2):
                wd0 = load_w(wd_d.ap()[bi, :, 0], 18)
                wd1 = load_w(wd_d.ap()[bi, :, 1], 18)
                out_d = out_tl if bi == 0 else out_br
                for sb in range(S // 2):
                    din = []
                    for ci in range(2):
                        t = hpool.tile([128, 10, 130], dt.bfloat16,
                                       tag=f"dls{ci}")
                        nc.sync.dma_start(
                            t[:],
                            tlb_d.ap()[bi, ci][:, 2 * _SR * sb:
                                               2 * _SR * sb + 10, :])
                        din.append(t)
                    for co, w in ((0, wd0), (1, wd1)):
                        ot2 = opool.tile([128, 2 * _SR, 128], dt.bfloat16,
                                         tag="ot")
                        for half in (0, 1):
                            off = half * _SR
                            ps = pspool.tile([128, 512], dt.float32, tag="ps")
                            i = 0
                            for ci in range(2):
                                for dy in range(3):
                                    for dx in range(3):
                                        nc.tensor.matmul(
                                            ps[:], w[:, ci * 9 + dy * 3 + dx],
                                            din[ci][:, off + dy:off + dy + _SR,
                                                    dx:dx + 128],
                                            start=(i == 0), stop=(i == 17))
                                        i += 1
                            act_half(ot2, half, ps, 8 + bi * 2 + co)
                        nc.sync.dma_start(
                            out_d.ap()[co * 128:(co + 1) * 128,
                                       2 * _SR * sb:2 * _SR * (sb + 1), :],
                            ot2[:])

    nc.compile()
    return nc


_NC_CACHE = {}


def _get_nc(H):
    if H not in _NC_CACHE:
        _NC_CACHE[H] = _build(H)
    return _NC_CACHE[H]


def kernel(**inputs):
    from concourse import bass_utils

    x = np.asarray(inputs["x"], np.float32)
    B, C, H, W = x.shape
    assert (C, W) == (256, 128) and H % _SR == 0

    shared = _prep_host(inputs)
    nc = _get_nc(H)

    in_maps = []
    for b in range(B):
        m = dict(shared)
        m["xpad"] = _pad_x_sample(x[b], H)
        in_maps.append(m)

    import os
    trace = bool(int(os.environ.get("KERNEL_TRACE", "0")))
    res = bass_utils.run_bass_kernel_spmd(
        nc, in_maps, core_ids=list(range(B)), trace=trace)
    kernel.last_result = res

    otl = np.stack([np.asarray(res.results[b]["out_tl"], _BF)
                    .astype(np.float32).reshape(256, H, 128)
                    for b in range(B)])
    obr = np.stack([np.asarray(res.results[b]["out_br"], _BF)
                    .astype(np.float32).reshape(256, H, 128)
                    for b in range(B)])
    return otl, obr
